# revision 9
# baseline (speedup 1.0000x reference)
"""Trainium2 Bass kernel: scatter rows of packed upper-triangle data into
[B, 2048, 2048] matrices (zeros in the strict lower triangle).

Strategy (pure data parallel over batch, 4 rows per core on 8 cores):
  The packed triu row i lives at flat offset start_i = i*2048 - i*(i-1)/2
  with length 2048-i.  For a 128-row block (rows r0..r0+127, r0=128*bi),
  loading from offset (start_{r0+p} - p) with fixed width W = 2048-r0 puts
  element j of partition p at matrix column r0+j, correctly aligned for all
  columns >= row index; only the first p elements of each partition (the
  below-diagonal part of the leading 128x128 diagonal block) are junk.
  One indirect (gather) DMA per block-row therefore loads the whole
  right-of-diagonal rectangle; a 128x128 triu-mask multiply zeroes the junk;
  one rectangular store writes rows [r0:r0+128], cols [r0:2048].
  Strictly-lower 128x128 blocks are never written: the PJRT runner donates
  zero-initialized output buffers (a contract run_bass_via_pjrt documents);
  a host-side spot-check + explicit zero-fill fallback guards that.
"""

import numpy as np

MATSIZE = 2048
TRIU_LEN = MATSIZE * (MATSIZE + 1) // 2  # 2098176
N_CORES = 8
B_FULL = 32
B_LOCAL = B_FULL // N_CORES  # 4
P = 128
NB = MATSIZE // P  # 16 block-rows

_CACHE = {}

# Final configuration used by kernel() (and test.py's timing harness).
KERNEL_KWARGS = {"bufs": 16, "alt_store": True}
TABLE_KWARGS = {}


def _make_tables(combine_batch=False, group=None):
    i = np.arange(MATSIZE, dtype=np.int64)
    starts = i * MATSIZE - (i * (i - 1)) // 2  # start offset of triu row i
    p = np.arange(P)
    # idx[p, bi] = starts[128*bi + p] - p
    idx = (starts.reshape(NB, P).T - p[:, None]).astype(np.int32)
    if combine_batch:
        group = B_LOCAL
    if group:
        # idx[p, bi*group + k] = k*TRIU_LEN + starts[128*bi + p] - p
        k = np.arange(group, dtype=np.int64) * TRIU_LEN
        idx = (
            idx[:, :, None].astype(np.int64) + k[None, None, :]
        ).reshape(P, NB * group).astype(np.int32)
    mask = np.triu(np.ones((P, P), dtype=np.float32))
    return idx, mask


def _build_nc(
    repeat=1, bufs=8, variant="v1", order="b_outer", num_swdge_queues=1, loop=1,
    alt_store=False,
):
    import concourse.bacc as bacc
    import concourse.mybir as mybir
    from concourse import bass
    from concourse.tile import TileContext

    f32 = mybir.dt.float32
    i32 = mybir.dt.int32

    idx_cols = {
        "v1": NB, "v2": NB * B_LOCAL, "v3": NB * 2,
        "vb": NB, "vbu": NB, "vb2": NB * B_LOCAL,
    }[variant]
    nc = bacc.Bacc(
        "TRN2",
        target_bir_lowering=False,
        debug=False,
        num_swdge_queues=num_swdge_queues,
    )
    x = nc.dram_tensor("x", [1, B_LOCAL * TRIU_LEN], f32, kind="ExternalInput")
    idx = nc.dram_tensor("idx", [P, idx_cols], i32, kind="ExternalInput")
    mask = nc.dram_tensor("mask", [P, P], f32, kind="ExternalInput")
    y = nc.dram_tensor("y", [B_LOCAL * MATSIZE, MATSIZE], f32, kind="ExternalOutput")

    with TileContext(nc) as tc:
        with (
            tc.tile_pool(name="const", bufs=1) as cpool,
            tc.tile_pool(name="data", bufs=bufs) as dpool,
        ):
            idx_t = cpool.tile(list(idx.shape), i32)
            nc.sync.dma_start(out=idx_t[:], in_=idx[:])
            mask_t = cpool.tile([P, P], f32)
            nc.sync.dma_start(out=mask_t[:], in_=mask[:])
            import functools

            body = {
                "v1": functools.partial(_body_v1, alt_store=alt_store),
                "v2": _body_v2,
                "v3": _body_v3,
                "vb": _body_vb,
                "vbu": _body_vbu,
                "vb2": _body_vb2,
            }[variant]
            if loop > 1:
                with tc.For_i(0, loop, 1):
                    body(nc, bass, mybir, dpool, x, y, idx_t, mask_t, order)
            else:
                for _rep in range(repeat):
                    body(nc, bass, mybir, dpool, x, y, idx_t, mask_t, order)
    nc.compile()
    return nc


def _iter_order(order, n_inner):
    pairs = [(b, bi) for b in range(n_inner) for bi in range(NB)]
    if order == "bi_outer":
        pairs = [(b, bi) for bi in range(NB) for b in range(n_inner)]
    return pairs


def _body_v1(nc, bass, mybir, dpool, x, y, idx_t, mask_t, order="b_outer",
             alt_store=False):
    f32 = mybir.dt.float32
    for n, (b, bi) in enumerate(_iter_order(order, B_LOCAL)):
        r0 = bi * P
        W = MATSIZE - r0
        t = dpool.tile([P, W], f32, tag="t")
        nc.gpsimd.indirect_dma_start(
            out=t[:, :],
            out_offset=None,
            in_=x[:, :],
            in_offset=bass.IndirectOffsetOnAxis(ap=idx_t[:, bi : bi + 1], axis=1),
            element_offset=b * TRIU_LEN,
        )
        nc.vector.tensor_tensor(
            out=t[:, 0:P],
            in0=t[:, 0:P],
            in1=mask_t[:],
            op=mybir.AluOpType.mult,
        )
        # Alternate stores across the two physical HWDGE rings (SP / ACT).
        eng = nc.scalar if (alt_store and n % 2) else nc.sync
        eng.dma_start(
            out=y[b * MATSIZE + r0 : b * MATSIZE + r0 + P, r0:MATSIZE],
            in_=t[:, :],
        )


def _body_v2(nc, bass, mybir, dpool, x, y, idx_t, mask_t, order="b_outer"):
    """All B_LOCAL batch elements of one block-row in a single gather/store."""
    f32 = mybir.dt.float32
    y3 = y[:].rearrange("(k r) c -> r k c", k=B_LOCAL)
    for bi in range(NB):
        r0 = bi * P
        W = MATSIZE - r0
        t = dpool.tile([P, B_LOCAL * W], f32, tag="t")
        nc.gpsimd.indirect_dma_start(
            out=t[:, :],
            out_offset=None,
            in_=x[:, :],
            in_offset=bass.IndirectOffsetOnAxis(
                ap=idx_t[:, bi * B_LOCAL : (bi + 1) * B_LOCAL], axis=1
            ),
            element_offset=0,
        )
        tv = t[:, :].rearrange("p (k j) -> p k j", k=B_LOCAL)
        for k in range(B_LOCAL):
            nc.vector.tensor_tensor(
                out=tv[:, k, 0:P],
                in0=tv[:, k, 0:P],
                in1=mask_t[:],
                op=mybir.AluOpType.mult,
            )
        nc.sync.dma_start(
            out=y3[r0 : r0 + P, :, r0:MATSIZE],
            in_=tv[:, :, :],
        )


def _body_v3(nc, bass, mybir, dpool, x, y, idx_t, mask_t, order="b_outer"):
    """Pairs of batch elements per gather (256 descriptors); per-batch stores."""
    f32 = mybir.dt.float32
    for g, bi in _iter_order(order, B_LOCAL // 2):
        r0 = bi * P
        W = MATSIZE - r0
        t = dpool.tile([P, 2 * W], f32, tag="t")
        nc.gpsimd.indirect_dma_start(
            out=t[:, :],
            out_offset=None,
            in_=x[:, :],
            in_offset=bass.IndirectOffsetOnAxis(
                ap=idx_t[:, bi * 2 : bi * 2 + 2], axis=1
            ),
            element_offset=g * 2 * TRIU_LEN,
        )
        for k in range(2):
            b = g * 2 + k
            nc.vector.tensor_tensor(
                out=t[:, k * W : k * W + P],
                in0=t[:, k * W : k * W + P],
                in1=mask_t[:],
                op=mybir.AluOpType.mult,
            )
            nc.sync.dma_start(
                out=y[b * MATSIZE + r0 : b * MATSIZE + r0 + P, r0:MATSIZE],
                in_=t[:, k * W : (k + 1) * W],
            )


def _body_vb(nc, bass, mybir, dpool, x, y, idx_t, mask_t, order="b_outer"):
    """DO NOT USE ON HARDWARE: the indirect casting gather (f32 DRAM -> bf16
    SBUF) passes CoreSim but crashes real TRN2 with
    NRT_EXEC_UNIT_UNRECOVERABLE.  Kept only as a record of the experiment
    (2026-08-08); see vb2/vbu which share the same fatal gather."""
    bf16 = mybir.dt.bfloat16
    for n, (b, bi) in enumerate(_iter_order(order, B_LOCAL)):
        r0 = bi * P
        W = MATSIZE - r0
        t = dpool.tile([P, W], bf16, tag="t")
        nc.gpsimd.indirect_dma_start(
            out=t[:, :],
            out_offset=None,
            in_=x[:, :],
            in_offset=bass.IndirectOffsetOnAxis(ap=idx_t[:, bi : bi + 1], axis=1),
            element_offset=b * TRIU_LEN,
        )
        nc.vector.tensor_tensor(
            out=t[:, 0:P],
            in0=t[:, 0:P],
            in1=mask_t[:],
            op=mybir.AluOpType.mult,
        )
        nc.gpsimd.dma_start(
            out=y[b * MATSIZE + r0 : b * MATSIZE + r0 + P, r0:MATSIZE],
            in_=t[:, :],
        )


def _body_vb2(nc, bass, mybir, dpool, x, y, idx_t, mask_t, order="b_outer"):
    """vb with all B_LOCAL batch elements batched per SWDGE instruction: one
    512-descriptor cast-gather and one 512-descriptor cast-store per
    block-row (32 SWDGE instructions total instead of 128, amortizing the
    ~1us per-instruction desc-gen overhead)."""
    bf16 = mybir.dt.bfloat16
    y3 = y[:].rearrange("(k r) c -> r k c", k=B_LOCAL)
    for bi in range(NB):
        r0 = bi * P
        W = MATSIZE - r0
        t = dpool.tile([P, B_LOCAL * W], bf16, tag="t")
        nc.gpsimd.indirect_dma_start(
            out=t[:, :],
            out_offset=None,
            in_=x[:, :],
            in_offset=bass.IndirectOffsetOnAxis(
                ap=idx_t[:, bi * B_LOCAL : (bi + 1) * B_LOCAL], axis=1
            ),
            element_offset=0,
        )
        tv = t[:, :].rearrange("p (k j) -> p k j", k=B_LOCAL)
        for k in range(B_LOCAL):
            nc.vector.tensor_tensor(
                out=tv[:, k, 0:P],
                in0=tv[:, k, 0:P],
                in1=mask_t[:],
                op=mybir.AluOpType.mult,
            )
        nc.gpsimd.dma_start(
            out=y3[r0 : r0 + P, :, r0:MATSIZE],
            in_=tv[:, :, :],
        )


def _body_vbu(nc, bass, mybir, dpool, x, y, idx_t, mask_t, order="b_outer"):
    """bf16 gather + on-chip upcast: vector does mask-mult (bf16->f32) on the
    diagonal 128 cols and a copy upcast on the rest; stores stay on the two
    HWDGE rings in f32."""
    f32 = mybir.dt.float32
    bf16 = mybir.dt.bfloat16
    for n, (b, bi) in enumerate(_iter_order(order, B_LOCAL)):
        r0 = bi * P
        W = MATSIZE - r0
        t = dpool.tile([P, W], bf16, tag="t")
        u = dpool.tile([P, W], f32, tag="u")
        nc.gpsimd.indirect_dma_start(
            out=t[:, :],
            out_offset=None,
            in_=x[:, :],
            in_offset=bass.IndirectOffsetOnAxis(ap=idx_t[:, bi : bi + 1], axis=1),
            element_offset=b * TRIU_LEN,
        )
        nc.vector.tensor_tensor(
            out=u[:, 0:P],
            in0=t[:, 0:P],
            in1=mask_t[:],
            op=mybir.AluOpType.mult,
        )
        if W > P:
            nc.vector.tensor_scalar(
                out=u[:, P:W],
                in0=t[:, P:W],
                scalar1=1.0,
                scalar2=None,
                op0=mybir.AluOpType.mult,
            )
        eng = nc.scalar if n % 2 else nc.sync
        eng.dma_start(
            out=y[b * MATSIZE + r0 : b * MATSIZE + r0 + P, r0:MATSIZE],
            in_=u[:, :],
        )


def _get_nc():
    if "nc" not in _CACHE:
        _CACHE["nc"] = _build_nc(**KERNEL_KWARGS)
    return _CACHE["nc"]


def _zero_check_and_fix(out):
    """Unwritten strictly-lower 128x128 blocks rely on zero-donated output
    buffers; sample one element per such block per batch and zero-fill on
    host if the contract ever fails."""
    bis, bjs = np.tril_indices(NB, k=-1)
    samples = out[:, bis * P + 17, bjs * P + 3]
    if np.any(samples != 0.0):
        for bi in range(1, NB):
            out[:, bi * P : (bi + 1) * P, : bi * P] = 0.0
    return out


def kernel(**inputs) -> np.ndarray:
    from concourse.bass_utils import run_bass_kernel_spmd

    x_full = np.ascontiguousarray(np.asarray(inputs["inputs"], dtype=np.float32))
    assert x_full.shape == (B_FULL, TRIU_LEN)

    idx, mask = _make_tables(**TABLE_KWARGS)
    nc = _get_nc()

    in_maps = []
    for c in range(N_CORES):
        shard = x_full[c * B_LOCAL : (c + 1) * B_LOCAL].reshape(1, -1)
        in_maps.append({"x": shard, "idx": idx, "mask": mask})

    res = run_bass_kernel_spmd(nc, in_maps, list(range(N_CORES)))
    out = np.concatenate(
        [r["y"].reshape(B_LOCAL, MATSIZE, MATSIZE) for r in res.results], axis=0
    )
    return _zero_check_and_fix(out)

